# revision 13
# baseline (speedup 1.0000x reference)
"""DGCNN-Vanilla encoder forward on 8 TRN2 NeuronCores (Bass/Tile).

Strategy: data-parallel over batch (2 batches per core). Per batch, per
128-point block: PE computes kNN scores s[n,m] = 2<x_n,x_m> - |x_m|^2 via a
4-dim homogeneous matmul; DVE extracts top-20 indices with 3 rounds of
max8/max_index/match_replace; SWDGE dma_gather fetches the 20 neighbor
feature rows (y = x@Wa, 64ch) per point; DVE reduces max/sum/sumsq over k.
BatchNorm statistics couple all 16 batches, so per-layer stats partials are
AllReduced (tiny payloads); the post-pool tail (linear layers on [16,1024])
is replicated on every core after one AllGather of pooled features + stats.

Monotonicity (bn gamma == 1 > 0) lets max-over-k / max-over-points commute
with bn+lrelu, so only pre-activation maxima are needed per point.
"""

import numpy as np

import concourse.bacc as bacc
import concourse.mybir as mybir
from concourse import tile
from concourse import bass_utils

f32 = mybir.dt.float32
u16 = mybir.dt.uint16
i16 = mybir.dt.int16

B, N, C, K = 16, 2048, 3, 20
NC_ = 8          # cores
BL = 2           # batches per core
NBLK = 16        # 128-point blocks per batch
P = 128
EPS = 1e-5
SLOPE = 0.2
AF = mybir.ActivationFunctionType
ALU = mybir.AluOpType
AX = mybir.AxisListType

TRACE = False  # set by test harness for profiling
DEBUG = False  # adds intermediate-dump outputs


def _build():
    nc = bacc.Bacc("TRN2", target_bir_lowering=False, debug=False, num_devices=NC_)

    # ---- inputs ----
    xq_ap = nc.dram_tensor("xq", [BL, 4, N], f32, kind="ExternalInput").ap()
    xp_ap = nc.dram_tensor("xp", [BL, 4, N], f32, kind="ExternalInput").ap()
    Wyc_ap = nc.dram_tensor("Wyc", [4, 128], f32, kind="ExternalInput").ap()
    W1_ap = nc.dram_tensor("W1", [64, 128], f32, kind="ExternalInput").ap()
    W2_ap = nc.dram_tensor("W2", [128, 128], f32, kind="ExternalInput").ap()
    W3_ap = nc.dram_tensor("W3", [128, 1024], f32, kind="ExternalInput").ap()
    W4_ap = nc.dram_tensor("W4", [128, 8, 512], f32, kind="ExternalInput").ap()
    W5_ap = nc.dram_tensor("W5", [128, 4, 512], f32, kind="ExternalInput").ap()
    W6_ap = nc.dram_tensor("W6", [128, 4, 1024], f32, kind="ExternalInput").ap()
    ob_ap = nc.dram_tensor("ob", [128, 8], f32, kind="ExternalInput").ap()
    bn_aps = {}
    for name, shp in [("bn0", [64, 2]), ("bn1", [128, 2]), ("bn2", [128, 2]),
                      ("bn3", [128, 8, 2]), ("bn4", [128, 4, 2]), ("bn5", [128, 4, 2])]:
        bn_aps[name] = nc.dram_tensor(name, shp, f32, kind="ExternalInput").ap()
    id_ap = nc.dram_tensor("ident", [128, 128], f32, kind="ExternalInput").ap()
    ones_ap = nc.dram_tensor("ones", [128, 1], f32, kind="ExternalInput").ap()
    out_ap = nc.dram_tensor("out", [16, 1024], f32, kind="ExternalOutput").ap()
    dbg = {}
    if DEBUG:
        for nm, shp, dt in [("dbg_s", [128, N], f32), ("dbg_i24", [128, 24], u16),
                            ("dbg_g", [128, K, 64], f32), ("dbg_z0T", [64, BL * N], f32),
                            ("dbg_red0", [64, 2], f32), ("dbg_h0T", [64, BL * N], f32),
                            ("dbg_red1", [128, 2], f32), ("dbg_pay3", [128, 8, 4], f32),
                            ("dbg_h3", [128, 8, 16], f32), ("dbg_c", [128, 64], f32),
                            ("dbg_maxa", [128, 64], f32)]:
            dbg[nm] = nc.dram_tensor(nm, shp, dt, kind="ExternalOutput").ap()

    groups = [list(range(NC_))]

    with tile.TileContext(nc) as tc:
        with (
            tc.tile_pool(name="glob", bufs=1) as G,
            tc.tile_pool(name="dram", bufs=1, space="DRAM") as DR,
            tc.tile_pool(name="dramrot", bufs=3, space="DRAM") as DRR,
        ):
            # ---- load persistent tensors ----
            def load(ap_in, shape, tag, dt=f32):
                t = G.tile(shape, dt, tag=tag, name=tag)
                nc.sync.dma_start(t[:], ap_in[:])
                return t

            xq = [load(xq_ap[bb], [4, N], f"xq{bb}") for bb in range(BL)]
            xp = [load(xp_ap[bb], [4, N], f"xp{bb}") for bb in range(BL)]
            Wyc = load(Wyc_ap, [4, 128], "Wyc")
            W1 = load(W1_ap, [64, 128], "W1")
            W2 = load(W2_ap, [128, 128], "W2")
            W3 = load(W3_ap, [128, 1024], "W3")
            W4 = load(W4_ap, [128, 8, 512], "W4")
            W5 = load(W5_ap, [128, 4, 512], "W5")
            W6 = load(W6_ap, [128, 4, 1024], "W6")
            obt = load(ob_ap, [128, 8], "ob")
            bn = {k: load(v, list(v.shape), k) for k, v in bn_aps.items()}
            ident = load(id_ap, [128, 128], "ident")
            ones = load(ones_ap, [128, 1], "ones")

            z0T = G.tile([64, BL * N], f32, tag="slotA", bufs=1, padded_shape=[128, BL * N])
            h0T = G.tile([64, BL * N], f32, tag="slotB", bufs=1, padded_shape=[128, BL * N])
            c_all = G.tile([128, BL * NBLK, 64], f32, tag="c_all")
            stat_sb = G.tile([64, 8], f32, tag="stat_sb")  # phase-A stats staging
            accs = G.tile([128, 5, 64], f32, tag="accs")  # suma, sumsq, t, c, csq
            y_dram = [DR.tile([N, 64], f32, tag=f"y{b}", name=f"y{b}") for b in range(BL)]

            # ---- collective helper ----
            def collective(src_tile_slice, shape, kind, tag):
                ib = DR.tile(shape, f32, tag=f"ib_{tag}", name=f"ib_{tag}")
                nc.sync.dma_start(ib[:], src_tile_slice)
                if kind == "AllReduce":
                    oshape = shape
                else:  # AllGather: concat on a new leading axis
                    oshape = [NC_] + shape
                obounce = DR.tile(oshape, f32, tag=f"ob_{tag}", name=f"ob_{tag}")
                nc.gpsimd.collective_compute(
                    kind,
                    ALU.add if kind == "AllReduce" else ALU.bypass,
                    replica_groups=groups,
                    ins=[ib.opt()],
                    outs=[obounce.opt()],
                )
                return obounce

            # warmup collective (absorbs CC cold start; overlaps with phase A)
            wu = DR.tile([1, 1], f32, tag="wu_i")
            nc.sync.dma_start(wu[:], ones[0:1, 0:1])
            wu_o = DR.tile([1, 1], f32, tag="wu_o")
            nc.gpsimd.collective_compute(
                "AllReduce", ALU.add, replica_groups=groups,
                ins=[wu.opt()], outs=[wu_o.opt()],
            )

            # =========================== PHASE A ===========================
            with (
                tc.tile_pool(name="pa", bufs=2) as PA,
                tc.tile_pool(name="pa_psum", bufs=1, space="PSUM") as PP,
                tc.tile_pool(name="pa_psum_sc", bufs=2, space="PSUM") as PSC,
            ):
                first_blk = [True]

                for b in range(BL):
                    # ---- prologue: y and c for the whole batch ----
                    for blk in range(NBLK):
                        yc = PP.tile([128, 128], f32, tag="yc")
                        nc.tensor.matmul(
                            yc[:], xq[b][:, blk * 128:(blk + 1) * 128], Wyc[:],
                            start=True, stop=True,
                        )
                        ybuf = PA.tile([128, 64], f32, tag="ybuf")
                        nc.scalar.activation(ybuf[:], yc[:, 0:64], AF.Copy)
                        nc.scalar.activation(c_all[:, b * NBLK + blk, :], yc[:, 64:128], AF.Copy)
                        nc.sync.dma_start(y_dram[b][blk * 128:(blk + 1) * 128, :], ybuf[:])

                for b in range(BL):
                    for blk in range(NBLK):
                        cb = c_all[:, b * NBLK + blk, :]
                        # ---- scores ----
                        s = PA.tile([128, N], f32, tag="s")
                        for half in range(2):
                            ps = PSC.tile([128, 1024], f32, tag="sc")
                            for j in range(2):
                                nc.tensor.matmul(
                                    ps[:, j * 512:(j + 1) * 512],
                                    xq[b][:, blk * 128:(blk + 1) * 128],
                                    xp[b][:, (half * 1024 + j * 512):(half * 1024 + (j + 1) * 512)],
                                    start=True, stop=True,
                                )
                            nc.scalar.activation(
                                s[:, half * 1024:(half + 1) * 1024], ps[:], AF.Copy
                            )
                        if DEBUG and b == 0 and blk == 0:
                            nc.sync.dma_start(dbg["dbg_s"][:], s[:])
                            nc.sync.dma_start(dbg["dbg_c"][:], cb)
                        # ---- top-20 ----
                        v24 = PA.tile([128, 24], f32, tag="v24")
                        i24 = PA.tile([128, 24], u16, tag="i24")
                        for r in range(3):
                            nc.vector.max(v24[:, 8 * r:8 * (r + 1)], s[:])
                            nc.vector.max_index(
                                i24[:, 8 * r:8 * (r + 1)], v24[:, 8 * r:8 * (r + 1)], s[:]
                            )
                            if r < 2:
                                nc.vector.match_replace(
                                    s[:], v24[:, 8 * r:8 * (r + 1)], s[:], -1e30
                                )
                        if DEBUG and b == 0 and blk == 0:
                            nc.sync.dma_start(dbg["dbg_i24"][:], i24[:])
                        # ---- idx repack: i24 -> D (flat j*128+p) -> wrapped idx_sb ----
                        D = DRR.tile([K * 128], u16, tag="D")
                        Dv = D[:]
                        for r, (j0, nj) in enumerate([(0, 8), (8, 8), (16, 4)]):
                            src = i24[:, j0:j0 + nj]
                            dst = Dv.rearrange("(j p) -> p j", p=128)[:, j0:j0 + nj]
                            nc.sync.dma_start(dst, src)
                        idx_sb = PA.tile([128, 8 * K], u16, tag="idx_sb")
                        # wrapped+replicated: idx_sb[16g+pp, s] = D[s*16+pp]
                        srcv = Dv.rearrange("(s pp) -> pp s", pp=16)  # [16, 160]
                        for g8 in range(8):
                            nc.sync.dma_start(idx_sb[16 * g8:16 * (g8 + 1), :], srcv)
                        # ---- gather ----
                        g = PA.tile([128, K, 64], f32, tag="g")
                        nc.gpsimd.dma_gather(
                            g[:], y_dram[b][:], idx_sb[:].bitcast(i16),
                            K * 128, K * 128, 64, single_packet=False,
                        )
                        if DEBUG and b == 0 and blk == 0:
                            nc.sync.dma_start(dbg["dbg_g"][:], g[:])
                        # ---- k reductions ----
                        gv = g[:].rearrange("p k c -> p c k")
                        maxa = PA.tile([128, 64], f32, tag="maxa")
                        suma = PA.tile([128, 64], f32, tag="suma")
                        nc.vector.tensor_reduce(maxa[:], gv, axis=AX.X, op=ALU.max)
                        nc.vector.tensor_reduce(suma[:], gv, axis=AX.X, op=ALU.add)
                        gsq = PA.tile([128, K, 64], f32, tag="gsq", bufs=1)
                        sq_acc = PA.tile([128, 1], f32, tag="sq_acc")
                        nc.scalar.activation(
                            gsq[:], g[:], AF.Square, accum_out=sq_acc[:]
                        )
                        sumsq = PA.tile([128, 64], f32, tag="sumsq")
                        nc.vector.tensor_reduce(
                            sumsq[:], gsq[:].rearrange("p k c -> p c k"), axis=AX.X, op=ALU.add
                        )
                        if DEBUG and b == 0 and blk == 0:
                            nc.sync.dma_start(dbg["dbg_maxa"][:], maxa[:])
                        # ---- z0 and stats contributions ----
                        z0 = PA.tile([128, 64], f32, tag="z0")
                        nc.vector.tensor_add(z0[:], maxa[:], cb)
                        t = PA.tile([128, 64], f32, tag="t")
                        nc.vector.tensor_mul(t[:], suma[:], cb)
                        csq = PA.tile([128, 64], f32, tag="csq")
                        nc.scalar.activation(csq[:], cb, AF.Square)
                        srcs = [suma[:], sumsq[:], t[:], cb, csq[:]]
                        if first_blk[0]:
                            first_blk[0] = False
                            for col, sap in enumerate(srcs):
                                nc.vector.tensor_copy(accs[:, col, :], sap)
                        else:
                            for col, sap in enumerate(srcs):
                                nc.vector.tensor_add(accs[:, col, :], accs[:, col, :], sap)
                        # ---- transpose z0 into z0T ----
                        zt = PP.tile([64, 128], f32, tag="zt")
                        nc.tensor.matmul(zt[:], z0[:], ident[:], is_transpose=True,
                                         start=True, stop=True)
                        nc.scalar.activation(
                            z0T[:, (b * N + blk * 128):(b * N + blk * 128 + 128)],
                            zt[:], AF.Copy,
                        )
                # partition-reduce the 5 accumulators: single matmuls vs ones
                for col in range(5):
                    pr = PSC.tile([64, 1], f32, tag="pr", name="pr", bufs=2)
                    nc.tensor.matmul(pr[:], accs[:, col, :], ones[:],
                                     start=True, stop=True)
                    nc.scalar.activation(stat_sb[:, col:col + 1], pr[:], AF.Copy)

            # ---- assemble BN0 stats partial: [64, 2] = (S1, S2) ----
            pay0 = G.tile([64, 2], f32, tag="pay0")
            # S1 = suma_sum + K * c_sum
            nc.vector.scalar_tensor_tensor(
                pay0[:, 0:1], stat_sb[:, 3:4], float(K), stat_sb[:, 0:1],
                op0=ALU.mult, op1=ALU.add,
            )
            # S2 = sumsq_sum + 2*t_sum + K*csq_sum
            tmp0 = G.tile([64, 1], f32, tag="tmp0")
            nc.vector.scalar_tensor_tensor(
                tmp0[:], stat_sb[:, 2:3], 2.0, stat_sb[:, 1:2],
                op0=ALU.mult, op1=ALU.add,
            )
            nc.vector.scalar_tensor_tensor(
                pay0[:, 1:2], stat_sb[:, 4:5], float(K), tmp0[:],
                op0=ALU.mult, op1=ALU.add,
            )
            ar0_o = collective(pay0[:], [64, 2], "AllReduce", "ar0")

            # ---- BN coeff helper ----
            def bn_coeffs(red_sb, g_col, beta_col, count, tag, pdim):
                """red_sb: [pdim, 2] summed (S1, S2). returns (a, bcoef) [pdim,1]."""
                mean = G.tile([pdim, 1], f32, tag=f"mean_{tag}", name=f"mean_{tag}")
                nc.vector.tensor_scalar_mul(mean[:], red_sb[:, 0:1], 1.0 / count)
                ex2 = G.tile([pdim, 1], f32, tag=f"ex2_{tag}", name=f"ex2_{tag}")
                nc.vector.tensor_scalar_mul(ex2[:], red_sb[:, 1:2], 1.0 / count)
                var = G.tile([pdim, 1], f32, tag=f"var_{tag}", name=f"var_{tag}")
                nc.vector.tensor_mul(var[:], mean[:], mean[:])
                nc.vector.tensor_sub(var[:], ex2[:], var[:])
                nc.vector.tensor_scalar_add(var[:], var[:], EPS)
                rec = G.tile([pdim, 1], f32, tag=f"rec_{tag}", name=f"rec_{tag}")
                nc.vector.reciprocal(rec[:], var[:])
                rs = G.tile([pdim, 1], f32, tag=f"rs_{tag}", name=f"rs_{tag}")
                nc.scalar.activation(rs[:], rec[:], AF.Sqrt)
                a = G.tile([pdim, 1], f32, tag=f"a_{tag}", name=f"a_{tag}")
                nc.vector.tensor_mul(a[:], g_col, rs[:])
                bc = G.tile([pdim, 1], f32, tag=f"b_{tag}", name=f"b_{tag}")
                nc.vector.tensor_mul(bc[:], a[:], mean[:])
                nc.vector.tensor_sub(bc[:], beta_col, bc[:])
                return a, bc

            red0 = G.tile([64, 2], f32, tag="red0")
            nc.sync.dma_start(red0[:], ar0_o[:])
            a0, b0 = bn_coeffs(red0, bn["bn0"][:, 0:1], bn["bn0"][:, 1:2],
                               B * N * K, "bn0", 64)
            if DEBUG:
                nc.sync.dma_start(dbg["dbg_red0"][:], red0[:])
                nc.sync.dma_start(dbg["dbg_z0T"][:], z0T[:])

            # =========================== PW PHASE ==========================
            with (
                tc.tile_pool(name="pc", bufs=2) as PC,
                tc.tile_pool(name="pc_psum", bufs=4, space="PSUM") as PZ,
            ):
                # BN0 apply + lrelu -> h0T ; accumulate h0 sums
                h0acc = G.tile([64, 8], f32, tag="h0acc", name="h0acc")
                for ch in range(8):
                    sl = slice(ch * 512, (ch + 1) * 512)
                    nc.scalar.activation(
                        h0T[:, sl], z0T[:, sl], AF.Identity, bias=b0[:], scale=a0[:],
                    )
                    nc.vector.scalar_tensor_tensor(
                        h0T[:, sl], h0T[:, sl], SLOPE, h0T[:, sl],
                        op0=ALU.mult, op1=ALU.max, accum_out=h0acc[:, ch:ch + 1],
                    )
                if DEBUG:
                    nc.sync.dma_start(dbg["dbg_h0T"][:], h0T[:])
                h0sum = G.tile([64, 1], f32, tag="h0sum", name="h0sum")
                nc.vector.tensor_reduce(h0sum[:], h0acc[:], axis=AX.X, op=ALU.add)

                def pw_layer(W_t, h_in, zsb, tag, hsum):
                    """z = W^T @ h_in -> zsb (SBUF); returns s12 [128,2] partials."""
                    s2acc = G.tile([128, 8], f32, tag=f"s2acc_{tag}", name=f"s2acc_{tag}")
                    junk = PC.tile([128, 512], f32, tag="junk", name="junk")
                    for ch in range(8):
                        zp = PZ.tile([128, 512], f32, tag="z", name="zp")
                        nc.tensor.matmul(
                            zp[:], W_t[:], h_in[:, ch * 512:(ch + 1) * 512],
                            start=True, stop=True,
                        )
                        nc.scalar.activation(
                            zsb[:, ch * 512:(ch + 1) * 512], zp[:], AF.Copy)
                        nc.scalar.activation(
                            junk[:], zp[:], AF.Square, accum_out=s2acc[:, ch:ch + 1]
                        )
                    s12 = G.tile([128, 2], f32, tag=f"s12_{tag}", name=f"s12_{tag}")
                    nc.vector.tensor_reduce(s12[:, 1:2], s2acc[:], axis=AX.X, op=ALU.add)
                    s1p = PZ.tile([128, 1], f32, tag="s1p", name="s1p", bufs=2)
                    nc.tensor.matmul(s1p[:], W_t[:], hsum[:], start=True, stop=True)
                    nc.scalar.activation(s12[:, 0:1], s1p[:], AF.Copy)
                    return s12

                def bn_apply(zsb, hout, a_, b_, pdim, tag):
                    hacc = G.tile([pdim, 8], f32, tag=f"hacc_{tag}", name=f"hacc_{tag}")
                    for ch in range(8):
                        sl = slice(ch * 512, (ch + 1) * 512)
                        nc.scalar.activation(
                            hout[:, sl], zsb[:, sl], AF.Identity, bias=b_[:], scale=a_[:],
                        )
                        nc.vector.scalar_tensor_tensor(
                            hout[:, sl], hout[:, sl], SLOPE, hout[:, sl],
                            op0=ALU.mult, op1=ALU.max, accum_out=hacc[:, ch:ch + 1],
                        )
                    hsum = G.tile([pdim, 1], f32, tag=f"hsum_{tag}", name=f"hsum_{tag}")
                    nc.vector.tensor_reduce(hsum[:], hacc[:], axis=AX.X, op=ALU.add)
                    return hsum

                # ---- pw1 ----
                z1sb = G.tile([128, BL * N], f32, tag="slotA", bufs=1, name="z1sb")
                s12_1 = pw_layer(W1, h0T, z1sb, "z1", h0sum)
                ar1_o = collective(s12_1[:], [128, 2], "AllReduce", "ar1")
                red1 = G.tile([128, 2], f32, tag="red1", name="red1")
                nc.sync.dma_start(red1[:], ar1_o[:])
                a1, b1 = bn_coeffs(red1, bn["bn1"][:, 0:1], bn["bn1"][:, 1:2],
                                   B * N, "bn1", 128)
                if DEBUG:
                    nc.sync.dma_start(dbg["dbg_red1"][:], red1[:])
                h1 = G.tile([128, BL * N], f32, tag="slotB", bufs=1, name="h1")
                h1sum = bn_apply(z1sb, h1, a1, b1, 128, "h1")

                # ---- pw2 ----
                z2sb = G.tile([128, BL * N], f32, tag="slotA", bufs=1, name="z2sb")
                s12_2 = pw_layer(W2, h1, z2sb, "z2", h1sum)
                ar2_o = collective(s12_2[:], [128, 2], "AllReduce", "ar2")
                red2 = G.tile([128, 2], f32, tag="red2", name="red2")
                nc.sync.dma_start(red2[:], ar2_o[:])
                a2, b2 = bn_coeffs(red2, bn["bn2"][:, 0:1], bn["bn2"][:, 1:2],
                                   B * N, "bn2", 128)
                h2 = G.tile([128, BL * N], f32, tag="slotB", bufs=1, name="h2")
                h2sum = bn_apply(z2sb, h2, a2, b2, 128, "h2")

                # ---- pw3 (1024 out-ch in 8 chunks) + pool-max per batch ----
                pay3 = G.tile([128, 8, 4], f32, tag="pay3", name="pay3")
                s2z3 = G.tile([128, 8, 8], f32, tag="s2z3", name="s2z3")
                pools = G.tile([128, 8, 8], f32, tag="pools", name="pools")
                junk3 = PC.tile([128, 512], f32, tag="junk3", name="junk3")
                for cc in range(8):
                    for pt in range(8):
                        zp = PZ.tile([128, 512], f32, tag="z", name="zp3")
                        nc.tensor.matmul(
                            zp[:], W3[:, cc * 128:(cc + 1) * 128],
                            h2[:, pt * 512:(pt + 1) * 512],
                            start=True, stop=True,
                        )
                        nc.scalar.activation(
                            junk3[:], zp[:], AF.Square,
                            accum_out=s2z3[:, cc, pt:pt + 1],
                        )
                        nc.vector.tensor_reduce(
                            pools[:, cc, pt:pt + 1], zp[:], axis=AX.X, op=ALU.max)
                    nc.vector.tensor_reduce(
                        pay3[:, cc, 0:1], pools[:, cc, 0:4], axis=AX.X, op=ALU.max)
                    nc.vector.tensor_reduce(
                        pay3[:, cc, 1:2], pools[:, cc, 4:8], axis=AX.X, op=ALU.max)
                    s1p3 = PZ.tile([128, 1], f32, tag="s1p", name="s1p3", bufs=2)
                    nc.tensor.matmul(s1p3[:], W3[:, cc * 128:(cc + 1) * 128],
                                     h2sum[:], start=True, stop=True)
                    nc.scalar.activation(pay3[:, cc, 2:3], s1p3[:], AF.Copy)
                    nc.vector.tensor_reduce(
                        pay3[:, cc, 3:4], s2z3[:, cc, :], axis=AX.X, op=ALU.add)

                if DEBUG:
                    nc.sync.dma_start(dbg["dbg_pay3"][:], pay3[:])
                ag_o = collective(pay3[:], [128, 8, 4], "AllGather", "ag")

                # ================== TAIL (replicated) ==================
                agg = G.tile([128, 8, NC_, 4], f32, tag="agg", name="agg")
                nc.sync.dma_start(
                    agg[:], ag_o[:].rearrange("core p cc col -> p cc core col")
                )
                red3 = G.tile([128, 8, 2], f32, tag="red3", name="red3")
                nc.vector.tensor_reduce(
                    red3[:], agg[:, :, :, 2:4].rearrange("p cc core col -> p cc col core"),
                    axis=AX.X, op=ALU.add,
                )
                pooled = G.tile([128, 8, 8, 2], f32, tag="pooled", name="pooled")
                nc.sync.dma_start(
                    pooled[:],
                    ag_o[:].rearrange("core p cc col -> p cc core col")[:, :, :, 0:2],
                )
                h3 = G.tile([128, 8, 16], f32, tag="h3", name="h3")
                for cc in range(8):
                    a3, b3 = bn_coeffs(
                        red3[:, cc, :], bn["bn3"][:, cc, 0:1], bn["bn3"][:, cc, 1:2],
                        B * N, f"bn3_{cc}", 128)
                    nc.scalar.activation(
                        h3[:, cc, :],
                        pooled[:].rearrange("p cc core col -> p cc (core col)")[:, cc, :],
                        AF.Identity, bias=b3[:], scale=a3[:],
                    )
                    nc.vector.scalar_tensor_tensor(
                        h3[:, cc, :], h3[:, cc, :], SLOPE, h3[:, cc, :],
                        op0=ALU.mult, op1=ALU.max,
                    )

                if DEBUG:
                    nc.sync.dma_start(dbg["dbg_h3"][:], h3[:])

                def lin_layer(Wt, h_in, kc_n, mc_n, bn_t, tag):
                    hout = G.tile([128, mc_n, 16], f32, tag=f"h_{tag}", name=f"h_{tag}")
                    for mc in range(mc_n):
                        zp = PZ.tile([128, 16], f32, tag="z", name=f"zl_{tag}")
                        for kc in range(kc_n):
                            nc.tensor.matmul(
                                zp[:], Wt[:, kc, mc * 128:(mc + 1) * 128],
                                h_in[:, kc, :],
                                start=(kc == 0), stop=(kc == kc_n - 1),
                            )
                        s12 = G.tile([128, 2], f32, tag=f"s12_{tag}", name=f"s12_{tag}")
                        zs = G.tile([128, 16], f32, tag=f"zs_{tag}", name=f"zs_{tag}")
                        nc.scalar.activation(zs[:], zp[:], AF.Square,
                                             accum_out=s12[:, 1:2])
                        zc = G.tile([128, 16], f32, tag=f"zc_{tag}", name=f"zc_{tag}")
                        nc.scalar.activation(zc[:], zp[:], AF.Copy)
                        nc.vector.tensor_reduce(s12[:, 0:1], zc[:], axis=AX.X, op=ALU.add)
                        a_, b_ = bn_coeffs(s12, bn_t[:, mc, 0:1], bn_t[:, mc, 1:2],
                                           B, f"{tag}_{mc}", 128)
                        nc.scalar.activation(
                            hout[:, mc, :], zc[:],
                            AF.Identity, bias=b_[:], scale=a_[:],
                        )
                        nc.vector.scalar_tensor_tensor(
                            hout[:, mc, :], hout[:, mc, :], SLOPE, hout[:, mc, :],
                            op0=ALU.mult, op1=ALU.max,
                        )
                    return hout

                h4 = lin_layer(W4, h3, 8, 4, bn["bn4"], "l4")
                h5 = lin_layer(W5, h4, 4, 4, bn["bn5"], "l5")

                osb = G.tile([16, 8, 128], f32, tag="osb", name="osb")
                for mc in range(8):
                    zp = PZ.tile([128, 16], f32, tag="z", name="zfin")
                    for kc in range(4):
                        nc.tensor.matmul(
                            zp[:], W6[:, kc, mc * 128:(mc + 1) * 128], h5[:, kc, :],
                            start=(kc == 0), stop=(kc == 3),
                        )
                    zb = G.tile([128, 16], f32, tag="zb", name="zb")
                    nc.scalar.activation(zb[:], zp[:], AF.Identity,
                                         bias=obt[:, mc:mc + 1], scale=1.0)
                    tp = PZ.tile([16, 128], f32, tag="z", name="ztp")
                    nc.tensor.matmul(tp[:], zb[:], ident[:], is_transpose=True,
                                     start=True, stop=True)
                    nc.scalar.activation(osb[:, mc, :], tp[:], AF.Copy)
                nc.sync.dma_start(out_ap[:], osb[:])

    nc.compile()
    return nc


def _prep_inputs(x, edge_W, edge_b, pw_Ws, lin_Ws, out_W, out_b,
                 edge_g, edge_beta, pw_gs, pw_betas, lin_gs, lin_betas):
    x = np.asarray(x, dtype=np.float32)
    shared = {}
    Wa = np.asarray(edge_W[:3], np.float32)           # [3, 64]
    Wd = np.asarray(edge_W[3:], np.float32) - Wa      # [3, 64]
    Wyc = np.zeros((4, 128), np.float32)
    Wyc[0:3, 0:64] = Wa
    Wyc[0:3, 64:128] = Wd
    Wyc[3, 64:128] = -np.asarray(edge_b, np.float32)
    shared["Wyc"] = Wyc
    shared["W1"] = np.ascontiguousarray(np.asarray(pw_Ws[0], np.float32))
    shared["W2"] = np.ascontiguousarray(np.asarray(pw_Ws[1], np.float32))
    shared["W3"] = np.ascontiguousarray(np.asarray(pw_Ws[2], np.float32))
    shared["W4"] = np.ascontiguousarray(
        np.asarray(lin_Ws[0], np.float32).reshape(8, 128, 512).transpose(1, 0, 2))
    shared["W5"] = np.ascontiguousarray(
        np.asarray(lin_Ws[1], np.float32).reshape(4, 128, 512).transpose(1, 0, 2))
    shared["W6"] = np.ascontiguousarray(
        np.asarray(out_W, np.float32).reshape(4, 128, 1024).transpose(1, 0, 2))
    shared["ob"] = np.ascontiguousarray(
        np.asarray(out_b, np.float32).reshape(8, 128).T)
    shared["bn0"] = np.stack([np.asarray(edge_g, np.float32),
                              np.asarray(edge_beta, np.float32)], axis=1)
    shared["bn1"] = np.stack([np.asarray(pw_gs[0], np.float32),
                              np.asarray(pw_betas[0], np.float32)], axis=1)
    shared["bn2"] = np.stack([np.asarray(pw_gs[1], np.float32),
                              np.asarray(pw_betas[1], np.float32)], axis=1)
    shared["bn3"] = np.ascontiguousarray(
        np.stack([np.asarray(pw_gs[2], np.float32),
                  np.asarray(pw_betas[2], np.float32)], axis=1)
        .reshape(8, 128, 2).transpose(1, 0, 2))
    shared["bn4"] = np.ascontiguousarray(
        np.stack([np.asarray(lin_gs[0], np.float32),
                  np.asarray(lin_betas[0], np.float32)], axis=1)
        .reshape(4, 128, 2).transpose(1, 0, 2))
    shared["bn5"] = np.ascontiguousarray(
        np.stack([np.asarray(lin_gs[1], np.float32),
                  np.asarray(lin_betas[1], np.float32)], axis=1)
        .reshape(4, 128, 2).transpose(1, 0, 2))
    shared["ident"] = np.eye(128, dtype=np.float32)
    shared["ones"] = np.ones((128, 1), np.float32)

    in_maps = []
    for c in range(NC_):
        xb = x[2 * c:2 * c + 2]                      # [2, N, 3]
        xT = xb.transpose(0, 2, 1)                   # [2, 3, N]
        xx = np.sum(xb * xb, axis=-1)                # [2, N]
        m = dict(shared)
        m["xq"] = np.ascontiguousarray(np.concatenate(
            [xT, -np.ones((BL, 1, N), np.float32)], axis=1).astype(np.float32))
        m["xp"] = np.ascontiguousarray(np.concatenate(
            [2.0 * xT, xx[:, None, :]], axis=1).astype(np.float32))
        in_maps.append(m)
    return in_maps


_NC_CACHE = [None]
_LAST_RESULT = [None]


def kernel(x, edge_W, edge_b, edge_g, edge_beta,
           pw_Ws, pw_bs, pw_gs, pw_betas,
           lin_Ws, lin_bs, lin_gs, lin_betas,
           out_W, out_b):
    # conv/linear biases before a training-mode BN cancel exactly (the BN mean
    # absorbs them); only edge_b (inside the max path, also cancels but kept)
    # and out_b (no BN after) matter.
    if _NC_CACHE[0] is None:
        _NC_CACHE[0] = _build()
    nc = _NC_CACHE[0]
    in_maps = _prep_inputs(x, edge_W, edge_b, pw_Ws, lin_Ws, out_W, out_b,
                           edge_g, edge_beta, pw_gs, pw_betas, lin_gs, lin_betas)
    res = bass_utils.run_bass_kernel_spmd(
        nc, in_maps, core_ids=list(range(NC_)), trace=TRACE
    )
    _LAST_RESULT[0] = res
    return np.asarray(res.results[0]["out"], dtype=np.float32)


# revision 14
# speedup vs baseline: 2.1384x; 2.1384x over previous
"""DGCNN-Vanilla encoder forward on 8 TRN2 NeuronCores (Bass/Tile).

Strategy: data-parallel over batch (2 batches per core). Per batch, per
128-point block: PE computes kNN scores s[n,m] = 2<x_n,x_m> - |x_m|^2 via a
4-dim homogeneous matmul; DVE extracts top-20 indices with 3 rounds of
max8/max_index/match_replace; SWDGE dma_gather fetches the 20 neighbor
feature rows (y = x@Wa, 64ch) per point; DVE reduces max/sum/sumsq over k.
BatchNorm statistics couple all 16 batches, so per-layer stats partials are
AllReduced (tiny payloads); the post-pool tail (linear layers on [16,1024])
is replicated on every core after one AllGather of pooled features + stats.

Monotonicity (bn gamma == 1 > 0) lets max-over-k / max-over-points commute
with bn+lrelu, so only pre-activation maxima are needed per point.
"""

import numpy as np

import concourse.bacc as bacc
import concourse.mybir as mybir
from concourse import tile
from concourse import bass_utils

f32 = mybir.dt.float32
u16 = mybir.dt.uint16
i16 = mybir.dt.int16

B, N, C, K = 16, 2048, 3, 20
NC_ = 8          # cores
BL = 2           # batches per core
NBLK = 16        # 128-point blocks per batch
P = 128
EPS = 1e-5
SLOPE = 0.2
AF = mybir.ActivationFunctionType
ALU = mybir.AluOpType
AX = mybir.AxisListType

TRACE = False  # set by test harness for profiling
DEBUG = False  # adds intermediate-dump outputs


def _build():
    nc = bacc.Bacc("TRN2", target_bir_lowering=False, debug=False, num_devices=NC_)

    # ---- inputs ----
    xq_ap = nc.dram_tensor("xq", [BL, 4, N], f32, kind="ExternalInput").ap()
    xp_ap = nc.dram_tensor("xp", [BL, 4, N], f32, kind="ExternalInput").ap()
    Wyc_ap = nc.dram_tensor("Wyc", [4, 128], f32, kind="ExternalInput").ap()
    W1_ap = nc.dram_tensor("W1", [64, 128], f32, kind="ExternalInput").ap()
    W2_ap = nc.dram_tensor("W2", [128, 128], f32, kind="ExternalInput").ap()
    W3_ap = nc.dram_tensor("W3", [128, 1024], f32, kind="ExternalInput").ap()
    W4_ap = nc.dram_tensor("W4", [128, 8, 512], f32, kind="ExternalInput").ap()
    W5_ap = nc.dram_tensor("W5", [128, 4, 512], f32, kind="ExternalInput").ap()
    W6_ap = nc.dram_tensor("W6", [128, 4, 1024], f32, kind="ExternalInput").ap()
    ob_ap = nc.dram_tensor("ob", [128, 8], f32, kind="ExternalInput").ap()
    bn_aps = {}
    for name, shp in [("bn0", [64, 2]), ("bn1", [128, 2]), ("bn2", [128, 2]),
                      ("bn3", [128, 8, 2]), ("bn4", [128, 4, 2]), ("bn5", [128, 4, 2])]:
        bn_aps[name] = nc.dram_tensor(name, shp, f32, kind="ExternalInput").ap()
    id_ap = nc.dram_tensor("ident", [128, 128], f32, kind="ExternalInput").ap()
    ones_ap = nc.dram_tensor("ones", [128, 1], f32, kind="ExternalInput").ap()
    out_ap = nc.dram_tensor("out", [16, 1024], f32, kind="ExternalOutput").ap()
    dbg = {}
    if DEBUG:
        for nm, shp, dt in [("dbg_s", [128, N], f32), ("dbg_i24", [128, 24], u16),
                            ("dbg_g", [128, K, 64], f32), ("dbg_z0T", [64, BL * N], f32),
                            ("dbg_red0", [64, 2], f32), ("dbg_h0T", [64, BL * N], f32),
                            ("dbg_red1", [128, 2], f32), ("dbg_pay3", [128, 8, 4], f32),
                            ("dbg_h3", [128, 8, 16], f32), ("dbg_c", [128, 64], f32),
                            ("dbg_maxa", [128, 64], f32)]:
            dbg[nm] = nc.dram_tensor(nm, shp, dt, kind="ExternalOutput").ap()

    groups = [list(range(NC_))]

    with tile.TileContext(nc) as tc:
        with (
            tc.tile_pool(name="glob", bufs=1) as G,
            tc.tile_pool(name="dram", bufs=1, space="DRAM") as DR,
            tc.tile_pool(name="dramrot", bufs=3, space="DRAM") as DRR,
        ):
            # ---- load persistent tensors ----
            def load(ap_in, shape, tag, dt=f32):
                t = G.tile(shape, dt, tag=tag, name=tag)
                nc.sync.dma_start(t[:], ap_in[:])
                return t

            xq = [load(xq_ap[bb], [4, N], f"xq{bb}") for bb in range(BL)]
            xp = [load(xp_ap[bb], [4, N], f"xp{bb}") for bb in range(BL)]
            Wyc = load(Wyc_ap, [4, 128], "Wyc")
            W1 = load(W1_ap, [64, 128], "W1")
            W2 = load(W2_ap, [128, 128], "W2")
            W3 = load(W3_ap, [128, 1024], "W3")
            W4 = load(W4_ap, [128, 8, 512], "W4")
            W5 = load(W5_ap, [128, 4, 512], "W5")
            W6 = load(W6_ap, [128, 4, 1024], "W6")
            obt = load(ob_ap, [128, 8], "ob")
            bn = {k: load(v, list(v.shape), k) for k, v in bn_aps.items()}
            ident = load(id_ap, [128, 128], "ident")
            ones = load(ones_ap, [128, 1], "ones")

            z0T = G.tile([64, BL * N], f32, tag="slotA", bufs=1, padded_shape=[128, BL * N])
            h0T = G.tile([64, BL * N], f32, tag="slotB", bufs=1, padded_shape=[128, BL * N])
            c_all = G.tile([128, BL * NBLK, 64], f32, tag="c_all")
            stat_sb = G.tile([64, 8], f32, tag="stat_sb")  # phase-A stats staging
            accs = G.tile([128, 5, 64], f32, tag="accs")  # suma, sumsq, t, c, csq
            y_dram = [DR.tile([N, 64], f32, tag=f"y{b}", name=f"y{b}") for b in range(BL)]

            # ---- collective helper ----
            def collective(src_tile_slice, shape, kind, tag):
                ib = DR.tile(shape, f32, tag=f"ib_{tag}", name=f"ib_{tag}")
                nc.sync.dma_start(ib[:], src_tile_slice)
                if kind == "AllReduce":
                    oshape = shape
                else:  # AllGather: concat on a new leading axis
                    oshape = [NC_] + shape
                obounce = DR.tile(oshape, f32, tag=f"ob_{tag}", name=f"ob_{tag}")
                nc.gpsimd.collective_compute(
                    kind,
                    ALU.add if kind == "AllReduce" else ALU.bypass,
                    replica_groups=groups,
                    ins=[ib.opt()],
                    outs=[obounce.opt()],
                )
                return obounce

            # warmup collective (absorbs CC cold start; overlaps with phase A)
            wu = DR.tile([1, 1], f32, tag="wu_i")
            nc.sync.dma_start(wu[:], ones[0:1, 0:1])
            wu_o = DR.tile([1, 1], f32, tag="wu_o")
            nc.gpsimd.collective_compute(
                "AllReduce", ALU.add, replica_groups=groups,
                ins=[wu.opt()], outs=[wu_o.opt()],
            )

            # =========================== PHASE A ===========================
            with (
                tc.tile_pool(name="pa", bufs=2) as PA,
                tc.tile_pool(name="pa_psum", bufs=1, space="PSUM") as PP,
                tc.tile_pool(name="pa_psum_sc", bufs=2, space="PSUM") as PSC,
            ):
                first_blk = [True]

                for b in range(BL):
                    # ---- prologue: y and c for the whole batch ----
                    for blk in range(NBLK):
                        yc = PP.tile([128, 128], f32, tag="yc")
                        nc.tensor.matmul(
                            yc[:], xq[b][:, blk * 128:(blk + 1) * 128], Wyc[:],
                            start=True, stop=True,
                        )
                        ybuf = PA.tile([128, 64], f32, tag="ybuf")
                        nc.scalar.activation(ybuf[:], yc[:, 0:64], AF.Copy)
                        nc.scalar.activation(c_all[:, b * NBLK + blk, :], yc[:, 64:128], AF.Copy)
                        nc.sync.dma_start(y_dram[b][blk * 128:(blk + 1) * 128, :], ybuf[:])

                for b in range(BL):
                    for blk in range(NBLK):
                        cb = c_all[:, b * NBLK + blk, :]
                        # ---- scores ----
                        s = PA.tile([128, N], f32, tag="s")
                        for half in range(2):
                            ps = PSC.tile([128, 1024], f32, tag="sc")
                            for j in range(2):
                                nc.tensor.matmul(
                                    ps[:, j * 512:(j + 1) * 512],
                                    xq[b][:, blk * 128:(blk + 1) * 128],
                                    xp[b][:, (half * 1024 + j * 512):(half * 1024 + (j + 1) * 512)],
                                    start=True, stop=True,
                                )
                            nc.scalar.activation(
                                s[:, half * 1024:(half + 1) * 1024], ps[:], AF.Copy
                            )
                        if DEBUG and b == 0 and blk == 0:
                            nc.sync.dma_start(dbg["dbg_s"][:], s[:])
                            nc.sync.dma_start(dbg["dbg_c"][:], cb)
                        # ---- top-20 ----
                        v24 = PA.tile([128, 24], f32, tag="v24")
                        i24 = PA.tile([128, 24], u16, tag="i24")
                        for r in range(3):
                            nc.vector.max(v24[:, 8 * r:8 * (r + 1)], s[:])
                            nc.vector.max_index(
                                i24[:, 8 * r:8 * (r + 1)], v24[:, 8 * r:8 * (r + 1)], s[:]
                            )
                            if r < 2:
                                nc.vector.match_replace(
                                    s[:], v24[:, 8 * r:8 * (r + 1)], s[:], -1e30
                                )
                        if DEBUG and b == 0 and blk == 0:
                            nc.sync.dma_start(dbg["dbg_i24"][:], i24[:])
                        # ---- idx repack via two-level PE transpose ----
                        # wrapped layout: idx_sb[16g+pp, 8j+pb] = i24[16pb+pp, j]
                        i24f = PA.tile([128, 24], f32, tag="i24f")
                        nc.vector.tensor_copy(i24f[:, 0:20], i24[:, 0:20])
                        tp1 = PP.tile([20, 128], f32, tag="zt", name="tp1")
                        nc.tensor.matmul(tp1[:], i24f[:, 0:20], ident[:],
                                         is_transpose=True, start=True, stop=True)
                        t1s = PA.tile([20, 128], f32, tag="t1s")
                        nc.scalar.activation(t1s[:], tp1[:], AF.Copy)
                        idx_sb = PA.tile([128, 8 * K], u16, tag="idx_sb")
                        iv = idx_sb[:].rearrange("p (j pb) -> p j pb", pb=8)
                        for pb in range(8):
                            tp2 = PP.tile([16, 20], f32, tag="tp2", name="tp2")
                            nc.tensor.matmul(
                                tp2[:], t1s[:, 16 * pb:16 * (pb + 1)], ident[0:20, 0:20],
                                is_transpose=True, start=True, stop=True)
                            nc.vector.tensor_copy(iv[0:16, :, pb], tp2[:])
                        for g8 in range(1, 8):
                            nc.sync.dma_start(idx_sb[16 * g8:16 * (g8 + 1), :],
                                              idx_sb[0:16, :])
                        # ---- gather ----
                        g = PA.tile([128, K, 64], f32, tag="g")
                        nc.gpsimd.dma_gather(
                            g[:], y_dram[b][:], idx_sb[:].bitcast(i16),
                            K * 128, K * 128, 64, single_packet=False,
                        )
                        if DEBUG and b == 0 and blk == 0:
                            nc.sync.dma_start(dbg["dbg_g"][:], g[:])
                        # ---- k reductions ----
                        gv = g[:].rearrange("p k c -> p c k")
                        maxa = PA.tile([128, 64], f32, tag="maxa")
                        suma = PA.tile([128, 64], f32, tag="suma")
                        nc.vector.tensor_reduce(maxa[:], gv, axis=AX.X, op=ALU.max)
                        nc.vector.tensor_reduce(suma[:], gv, axis=AX.X, op=ALU.add)
                        gsq = PA.tile([128, K, 64], f32, tag="gsq", bufs=1)
                        sq_acc = PA.tile([128, 1], f32, tag="sq_acc")
                        nc.scalar.activation(
                            gsq[:], g[:], AF.Square, accum_out=sq_acc[:]
                        )
                        sumsq = PA.tile([128, 64], f32, tag="sumsq")
                        nc.vector.tensor_reduce(
                            sumsq[:], gsq[:].rearrange("p k c -> p c k"), axis=AX.X, op=ALU.add
                        )
                        if DEBUG and b == 0 and blk == 0:
                            nc.sync.dma_start(dbg["dbg_maxa"][:], maxa[:])
                        # ---- z0 and stats contributions ----
                        z0 = PA.tile([128, 64], f32, tag="z0")
                        nc.vector.tensor_add(z0[:], maxa[:], cb)
                        t = PA.tile([128, 64], f32, tag="t")
                        nc.vector.tensor_mul(t[:], suma[:], cb)
                        csq = PA.tile([128, 64], f32, tag="csq")
                        nc.scalar.activation(csq[:], cb, AF.Square)
                        srcs = [suma[:], sumsq[:], t[:], cb, csq[:]]
                        if first_blk[0]:
                            first_blk[0] = False
                            for col, sap in enumerate(srcs):
                                nc.vector.tensor_copy(accs[:, col, :], sap)
                        else:
                            for col, sap in enumerate(srcs):
                                nc.vector.tensor_add(accs[:, col, :], accs[:, col, :], sap)
                        # ---- transpose z0 into z0T ----
                        zt = PP.tile([64, 128], f32, tag="zt")
                        nc.tensor.matmul(zt[:], z0[:], ident[:], is_transpose=True,
                                         start=True, stop=True)
                        nc.scalar.activation(
                            z0T[:, (b * N + blk * 128):(b * N + blk * 128 + 128)],
                            zt[:], AF.Copy,
                        )
                # partition-reduce the 5 accumulators: single matmuls vs ones
                for col in range(5):
                    pr = PP.tile([64, 1], f32, tag="yc", name="pr")
                    nc.tensor.matmul(pr[:], accs[:, col, :], ones[:],
                                     start=True, stop=True)
                    nc.scalar.activation(stat_sb[:, col:col + 1], pr[:], AF.Copy)

            # ---- assemble BN0 stats partial: [64, 2] = (S1, S2) ----
            pay0 = G.tile([64, 2], f32, tag="pay0")
            # S1 = suma_sum + K * c_sum
            nc.vector.scalar_tensor_tensor(
                pay0[:, 0:1], stat_sb[:, 3:4], float(K), stat_sb[:, 0:1],
                op0=ALU.mult, op1=ALU.add,
            )
            # S2 = sumsq_sum + 2*t_sum + K*csq_sum
            tmp0 = G.tile([64, 1], f32, tag="tmp0")
            nc.vector.scalar_tensor_tensor(
                tmp0[:], stat_sb[:, 2:3], 2.0, stat_sb[:, 1:2],
                op0=ALU.mult, op1=ALU.add,
            )
            nc.vector.scalar_tensor_tensor(
                pay0[:, 1:2], stat_sb[:, 4:5], float(K), tmp0[:],
                op0=ALU.mult, op1=ALU.add,
            )
            ar0_o = collective(pay0[:], [64, 2], "AllReduce", "ar0")

            # ---- BN coeff helper ----
            def bn_coeffs(red_sb, g_col, beta_col, count, tag, pdim):
                """red_sb: [pdim, 2] summed (S1, S2). returns (a, bcoef) [pdim,1]."""
                mean = G.tile([pdim, 1], f32, tag=f"mean_{tag}", name=f"mean_{tag}")
                nc.vector.tensor_scalar_mul(mean[:], red_sb[:, 0:1], 1.0 / count)
                ex2 = G.tile([pdim, 1], f32, tag=f"ex2_{tag}", name=f"ex2_{tag}")
                nc.vector.tensor_scalar_mul(ex2[:], red_sb[:, 1:2], 1.0 / count)
                var = G.tile([pdim, 1], f32, tag=f"var_{tag}", name=f"var_{tag}")
                nc.vector.tensor_mul(var[:], mean[:], mean[:])
                nc.vector.tensor_sub(var[:], ex2[:], var[:])
                nc.vector.tensor_scalar_add(var[:], var[:], EPS)
                rec = G.tile([pdim, 1], f32, tag=f"rec_{tag}", name=f"rec_{tag}")
                nc.vector.reciprocal(rec[:], var[:])
                rs = G.tile([pdim, 1], f32, tag=f"rs_{tag}", name=f"rs_{tag}")
                nc.scalar.activation(rs[:], rec[:], AF.Sqrt)
                a = G.tile([pdim, 1], f32, tag=f"a_{tag}", name=f"a_{tag}")
                nc.vector.tensor_mul(a[:], g_col, rs[:])
                bc = G.tile([pdim, 1], f32, tag=f"b_{tag}", name=f"b_{tag}")
                nc.vector.tensor_mul(bc[:], a[:], mean[:])
                nc.vector.tensor_sub(bc[:], beta_col, bc[:])
                return a, bc

            red0 = G.tile([64, 2], f32, tag="red0")
            nc.sync.dma_start(red0[:], ar0_o[:])
            a0, b0 = bn_coeffs(red0, bn["bn0"][:, 0:1], bn["bn0"][:, 1:2],
                               B * N * K, "bn0", 64)
            if DEBUG:
                nc.sync.dma_start(dbg["dbg_red0"][:], red0[:])
                nc.sync.dma_start(dbg["dbg_z0T"][:], z0T[:])

            # =========================== PW PHASE ==========================
            with (
                tc.tile_pool(name="pc", bufs=2) as PC,
                tc.tile_pool(name="pc_psum", bufs=4, space="PSUM") as PZ,
            ):
                # BN0 apply + lrelu -> h0T ; accumulate h0 sums
                h0acc = G.tile([64, 8], f32, tag="h0acc", name="h0acc")
                for ch in range(8):
                    sl = slice(ch * 512, (ch + 1) * 512)
                    nc.scalar.activation(
                        h0T[:, sl], z0T[:, sl], AF.Identity, bias=b0[:], scale=a0[:],
                    )
                    nc.vector.scalar_tensor_tensor(
                        h0T[:, sl], h0T[:, sl], SLOPE, h0T[:, sl],
                        op0=ALU.mult, op1=ALU.max, accum_out=h0acc[:, ch:ch + 1],
                    )
                if DEBUG:
                    nc.sync.dma_start(dbg["dbg_h0T"][:], h0T[:])
                h0sum = G.tile([64, 1], f32, tag="h0sum", name="h0sum")
                nc.vector.tensor_reduce(h0sum[:], h0acc[:], axis=AX.X, op=ALU.add)

                def pw_layer(W_t, h_in, zsb, tag, hsum):
                    """z = W^T @ h_in -> zsb (SBUF); returns s12 [128,2] partials."""
                    s2acc = G.tile([128, 8], f32, tag=f"s2acc_{tag}", name=f"s2acc_{tag}")
                    junk = PC.tile([128, 512], f32, tag="junk", name="junk")
                    for ch in range(8):
                        zp = PZ.tile([128, 512], f32, tag="z", name="zp")
                        nc.tensor.matmul(
                            zp[:], W_t[:], h_in[:, ch * 512:(ch + 1) * 512],
                            start=True, stop=True,
                        )
                        nc.scalar.activation(
                            zsb[:, ch * 512:(ch + 1) * 512], zp[:], AF.Copy)
                        nc.scalar.activation(
                            junk[:], zp[:], AF.Square, accum_out=s2acc[:, ch:ch + 1]
                        )
                    s12 = G.tile([128, 2], f32, tag=f"s12_{tag}", name=f"s12_{tag}")
                    nc.vector.tensor_reduce(s12[:, 1:2], s2acc[:], axis=AX.X, op=ALU.add)
                    s1p = PZ.tile([128, 1], f32, tag="s1p", name="s1p", bufs=2)
                    nc.tensor.matmul(s1p[:], W_t[:], hsum[:], start=True, stop=True)
                    nc.scalar.activation(s12[:, 0:1], s1p[:], AF.Copy)
                    return s12

                def bn_apply(zsb, hout, a_, b_, pdim, tag):
                    hacc = G.tile([pdim, 8], f32, tag=f"hacc_{tag}", name=f"hacc_{tag}")
                    for ch in range(8):
                        sl = slice(ch * 512, (ch + 1) * 512)
                        nc.scalar.activation(
                            hout[:, sl], zsb[:, sl], AF.Identity, bias=b_[:], scale=a_[:],
                        )
                        nc.vector.scalar_tensor_tensor(
                            hout[:, sl], hout[:, sl], SLOPE, hout[:, sl],
                            op0=ALU.mult, op1=ALU.max, accum_out=hacc[:, ch:ch + 1],
                        )
                    hsum = G.tile([pdim, 1], f32, tag=f"hsum_{tag}", name=f"hsum_{tag}")
                    nc.vector.tensor_reduce(hsum[:], hacc[:], axis=AX.X, op=ALU.add)
                    return hsum

                # ---- pw1 ----
                z1sb = G.tile([128, BL * N], f32, tag="slotA", bufs=1, name="z1sb")
                s12_1 = pw_layer(W1, h0T, z1sb, "z1", h0sum)
                ar1_o = collective(s12_1[:], [128, 2], "AllReduce", "ar1")
                red1 = G.tile([128, 2], f32, tag="red1", name="red1")
                nc.sync.dma_start(red1[:], ar1_o[:])
                a1, b1 = bn_coeffs(red1, bn["bn1"][:, 0:1], bn["bn1"][:, 1:2],
                                   B * N, "bn1", 128)
                if DEBUG:
                    nc.sync.dma_start(dbg["dbg_red1"][:], red1[:])
                h1 = G.tile([128, BL * N], f32, tag="slotB", bufs=1, name="h1")
                h1sum = bn_apply(z1sb, h1, a1, b1, 128, "h1")

                # ---- pw2 ----
                z2sb = G.tile([128, BL * N], f32, tag="slotA", bufs=1, name="z2sb")
                s12_2 = pw_layer(W2, h1, z2sb, "z2", h1sum)
                ar2_o = collective(s12_2[:], [128, 2], "AllReduce", "ar2")
                red2 = G.tile([128, 2], f32, tag="red2", name="red2")
                nc.sync.dma_start(red2[:], ar2_o[:])
                a2, b2 = bn_coeffs(red2, bn["bn2"][:, 0:1], bn["bn2"][:, 1:2],
                                   B * N, "bn2", 128)
                h2 = G.tile([128, BL * N], f32, tag="slotB", bufs=1, name="h2")
                h2sum = bn_apply(z2sb, h2, a2, b2, 128, "h2")

                # ---- pw3 (1024 out-ch in 8 chunks) + pool-max per batch ----
                pay3 = G.tile([128, 8, 4], f32, tag="pay3", name="pay3")
                s2z3 = G.tile([128, 8, 8], f32, tag="s2z3", name="s2z3")
                pools = G.tile([128, 8, 8], f32, tag="pools", name="pools")
                junk3 = PC.tile([128, 512], f32, tag="junk3", name="junk3")
                for cc in range(8):
                    for pt in range(8):
                        zp = PZ.tile([128, 512], f32, tag="z", name="zp3")
                        nc.tensor.matmul(
                            zp[:], W3[:, cc * 128:(cc + 1) * 128],
                            h2[:, pt * 512:(pt + 1) * 512],
                            start=True, stop=True,
                        )
                        nc.scalar.activation(
                            junk3[:], zp[:], AF.Square,
                            accum_out=s2z3[:, cc, pt:pt + 1],
                        )
                        nc.vector.tensor_reduce(
                            pools[:, cc, pt:pt + 1], zp[:], axis=AX.X, op=ALU.max)
                    nc.vector.tensor_reduce(
                        pay3[:, cc, 0:1], pools[:, cc, 0:4], axis=AX.X, op=ALU.max)
                    nc.vector.tensor_reduce(
                        pay3[:, cc, 1:2], pools[:, cc, 4:8], axis=AX.X, op=ALU.max)
                    s1p3 = PZ.tile([128, 1], f32, tag="s1p", name="s1p3", bufs=2)
                    nc.tensor.matmul(s1p3[:], W3[:, cc * 128:(cc + 1) * 128],
                                     h2sum[:], start=True, stop=True)
                    nc.scalar.activation(pay3[:, cc, 2:3], s1p3[:], AF.Copy)
                    nc.vector.tensor_reduce(
                        pay3[:, cc, 3:4], s2z3[:, cc, :], axis=AX.X, op=ALU.add)

                if DEBUG:
                    nc.sync.dma_start(dbg["dbg_pay3"][:], pay3[:])
                ag_o = collective(pay3[:], [128, 8, 4], "AllGather", "ag")

                # ================== TAIL (replicated) ==================
                agg = G.tile([128, 8, NC_, 4], f32, tag="agg", name="agg")
                nc.sync.dma_start(
                    agg[:], ag_o[:].rearrange("core p cc col -> p cc core col")
                )
                red3 = G.tile([128, 8, 2], f32, tag="red3", name="red3")
                nc.vector.tensor_reduce(
                    red3[:], agg[:, :, :, 2:4].rearrange("p cc core col -> p cc col core"),
                    axis=AX.X, op=ALU.add,
                )
                pooled = G.tile([128, 8, 8, 2], f32, tag="pooled", name="pooled")
                nc.sync.dma_start(
                    pooled[:],
                    ag_o[:].rearrange("core p cc col -> p cc core col")[:, :, :, 0:2],
                )
                h3 = G.tile([128, 8, 16], f32, tag="h3", name="h3")
                for cc in range(8):
                    a3, b3 = bn_coeffs(
                        red3[:, cc, :], bn["bn3"][:, cc, 0:1], bn["bn3"][:, cc, 1:2],
                        B * N, f"bn3_{cc}", 128)
                    nc.scalar.activation(
                        h3[:, cc, :],
                        pooled[:].rearrange("p cc core col -> p cc (core col)")[:, cc, :],
                        AF.Identity, bias=b3[:], scale=a3[:],
                    )
                    nc.vector.scalar_tensor_tensor(
                        h3[:, cc, :], h3[:, cc, :], SLOPE, h3[:, cc, :],
                        op0=ALU.mult, op1=ALU.max,
                    )

                if DEBUG:
                    nc.sync.dma_start(dbg["dbg_h3"][:], h3[:])

                def lin_layer(Wt, h_in, kc_n, mc_n, bn_t, tag):
                    hout = G.tile([128, mc_n, 16], f32, tag=f"h_{tag}", name=f"h_{tag}")
                    for mc in range(mc_n):
                        zp = PZ.tile([128, 16], f32, tag="z", name=f"zl_{tag}")
                        for kc in range(kc_n):
                            nc.tensor.matmul(
                                zp[:], Wt[:, kc, mc * 128:(mc + 1) * 128],
                                h_in[:, kc, :],
                                start=(kc == 0), stop=(kc == kc_n - 1),
                            )
                        s12 = G.tile([128, 2], f32, tag=f"s12_{tag}", name=f"s12_{tag}")
                        zs = G.tile([128, 16], f32, tag=f"zs_{tag}", name=f"zs_{tag}")
                        nc.scalar.activation(zs[:], zp[:], AF.Square,
                                             accum_out=s12[:, 1:2])
                        zc = G.tile([128, 16], f32, tag=f"zc_{tag}", name=f"zc_{tag}")
                        nc.scalar.activation(zc[:], zp[:], AF.Copy)
                        nc.vector.tensor_reduce(s12[:, 0:1], zc[:], axis=AX.X, op=ALU.add)
                        a_, b_ = bn_coeffs(s12, bn_t[:, mc, 0:1], bn_t[:, mc, 1:2],
                                           B, f"{tag}_{mc}", 128)
                        nc.scalar.activation(
                            hout[:, mc, :], zc[:],
                            AF.Identity, bias=b_[:], scale=a_[:],
                        )
                        nc.vector.scalar_tensor_tensor(
                            hout[:, mc, :], hout[:, mc, :], SLOPE, hout[:, mc, :],
                            op0=ALU.mult, op1=ALU.max,
                        )
                    return hout

                h4 = lin_layer(W4, h3, 8, 4, bn["bn4"], "l4")
                h5 = lin_layer(W5, h4, 4, 4, bn["bn5"], "l5")

                osb = G.tile([16, 8, 128], f32, tag="osb", name="osb")
                for mc in range(8):
                    zp = PZ.tile([128, 16], f32, tag="z", name="zfin")
                    for kc in range(4):
                        nc.tensor.matmul(
                            zp[:], W6[:, kc, mc * 128:(mc + 1) * 128], h5[:, kc, :],
                            start=(kc == 0), stop=(kc == 3),
                        )
                    zb = G.tile([128, 16], f32, tag="zb", name="zb")
                    nc.scalar.activation(zb[:], zp[:], AF.Identity,
                                         bias=obt[:, mc:mc + 1], scale=1.0)
                    tp = PZ.tile([16, 128], f32, tag="z", name="ztp")
                    nc.tensor.matmul(tp[:], zb[:], ident[:], is_transpose=True,
                                     start=True, stop=True)
                    nc.scalar.activation(osb[:, mc, :], tp[:], AF.Copy)
                nc.sync.dma_start(out_ap[:], osb[:])

    nc.compile()
    return nc


def _prep_inputs(x, edge_W, edge_b, pw_Ws, lin_Ws, out_W, out_b,
                 edge_g, edge_beta, pw_gs, pw_betas, lin_gs, lin_betas):
    x = np.asarray(x, dtype=np.float32)
    shared = {}
    Wa = np.asarray(edge_W[:3], np.float32)           # [3, 64]
    Wd = np.asarray(edge_W[3:], np.float32) - Wa      # [3, 64]
    Wyc = np.zeros((4, 128), np.float32)
    Wyc[0:3, 0:64] = Wa
    Wyc[0:3, 64:128] = Wd
    Wyc[3, 64:128] = -np.asarray(edge_b, np.float32)
    shared["Wyc"] = Wyc
    shared["W1"] = np.ascontiguousarray(np.asarray(pw_Ws[0], np.float32))
    shared["W2"] = np.ascontiguousarray(np.asarray(pw_Ws[1], np.float32))
    shared["W3"] = np.ascontiguousarray(np.asarray(pw_Ws[2], np.float32))
    shared["W4"] = np.ascontiguousarray(
        np.asarray(lin_Ws[0], np.float32).reshape(8, 128, 512).transpose(1, 0, 2))
    shared["W5"] = np.ascontiguousarray(
        np.asarray(lin_Ws[1], np.float32).reshape(4, 128, 512).transpose(1, 0, 2))
    shared["W6"] = np.ascontiguousarray(
        np.asarray(out_W, np.float32).reshape(4, 128, 1024).transpose(1, 0, 2))
    shared["ob"] = np.ascontiguousarray(
        np.asarray(out_b, np.float32).reshape(8, 128).T)
    shared["bn0"] = np.stack([np.asarray(edge_g, np.float32),
                              np.asarray(edge_beta, np.float32)], axis=1)
    shared["bn1"] = np.stack([np.asarray(pw_gs[0], np.float32),
                              np.asarray(pw_betas[0], np.float32)], axis=1)
    shared["bn2"] = np.stack([np.asarray(pw_gs[1], np.float32),
                              np.asarray(pw_betas[1], np.float32)], axis=1)
    shared["bn3"] = np.ascontiguousarray(
        np.stack([np.asarray(pw_gs[2], np.float32),
                  np.asarray(pw_betas[2], np.float32)], axis=1)
        .reshape(8, 128, 2).transpose(1, 0, 2))
    shared["bn4"] = np.ascontiguousarray(
        np.stack([np.asarray(lin_gs[0], np.float32),
                  np.asarray(lin_betas[0], np.float32)], axis=1)
        .reshape(4, 128, 2).transpose(1, 0, 2))
    shared["bn5"] = np.ascontiguousarray(
        np.stack([np.asarray(lin_gs[1], np.float32),
                  np.asarray(lin_betas[1], np.float32)], axis=1)
        .reshape(4, 128, 2).transpose(1, 0, 2))
    shared["ident"] = np.eye(128, dtype=np.float32)
    shared["ones"] = np.ones((128, 1), np.float32)

    in_maps = []
    for c in range(NC_):
        xb = x[2 * c:2 * c + 2]                      # [2, N, 3]
        xT = xb.transpose(0, 2, 1)                   # [2, 3, N]
        xx = np.sum(xb * xb, axis=-1)                # [2, N]
        m = dict(shared)
        m["xq"] = np.ascontiguousarray(np.concatenate(
            [xT, -np.ones((BL, 1, N), np.float32)], axis=1).astype(np.float32))
        m["xp"] = np.ascontiguousarray(np.concatenate(
            [2.0 * xT, xx[:, None, :]], axis=1).astype(np.float32))
        in_maps.append(m)
    return in_maps


_NC_CACHE = [None]
_LAST_RESULT = [None]


def kernel(x, edge_W, edge_b, edge_g, edge_beta,
           pw_Ws, pw_bs, pw_gs, pw_betas,
           lin_Ws, lin_bs, lin_gs, lin_betas,
           out_W, out_b):
    # conv/linear biases before a training-mode BN cancel exactly (the BN mean
    # absorbs them); only edge_b (inside the max path, also cancels but kept)
    # and out_b (no BN after) matter.
    if _NC_CACHE[0] is None:
        _NC_CACHE[0] = _build()
    nc = _NC_CACHE[0]
    in_maps = _prep_inputs(x, edge_W, edge_b, pw_Ws, lin_Ws, out_W, out_b,
                           edge_g, edge_beta, pw_gs, pw_betas, lin_gs, lin_betas)
    res = bass_utils.run_bass_kernel_spmd(
        nc, in_maps, core_ids=list(range(NC_)), trace=TRACE
    )
    _LAST_RESULT[0] = res
    return np.asarray(res.results[0]["out"], dtype=np.float32)
